# revision 14
# baseline (speedup 1.0000x reference)
"""Single-head attention block (Q/K/V/O projections + softmax attention) on
8 Trainium2 NeuronCores.

Problem: x [16, 2048, 512] fp32; four 512x512 projections (torch convention
y = x @ W.T + b); scores = Q @ K.T / sqrt(512); softmax over keys;
out = attn @ V; y = out @ Wo.T + bo.

Sharding: pure data-parallel over batch -- each of the 8 cores computes 2 of
the 16 batches end-to-end. No collectives.

Algebraic restructuring (softmax is invariant to adding any function of the
query row, so those terms are dropped):
  scores ~ x A x^T + w[k]   with A = Wq^T Wk / sqrt(D), w = x (Wk^T bq)/sqrt(D)
  y = attn x B / rowsum + c with B = Wv^T Wo^T, c = bv Wo^T + bo
This removes the Q, K and V projections entirely.

Mixed-precision engine assignment (rel-err gate is 2e-2; this lands ~1.3e-2):
  * scores path runs fp8(e4m3) DoubleRow matmuls (256-deep contraction, 2x PE
    rate): HT[d',q] = (Ah+Al)-pairs.T @ xT8 / 16, scoresT = xT8-pairs.T @ HT.
    A is stored as fp8 hi (Ah = fp8(256*SCALE*WqTWk)) plus an unscaled fp8
    residual Al (error feedback) sharing the same PSUM accumulation; HT is
    evicted as fp8(psum/16) so every fp8 tensor sits in e4m3's normal range
    (max +-240). exp un-scales via the ACT scale operand: a = exp(ps/16 + w).
  * attention weights a, the ZT = x^T a accumulation, and the output
    projection run bf16 (1x PE rate, ~0.2% noise): fp8 quantization of a / x
    / ZT / B costs ~9e-3 rel err each, which would blow the error budget,
    while scores-side fp8 noise is softened by softmax + averaging.
x is resident in both layouts: xT8 fp8 [128, ND*S] (PE transpose of the DMA
tiles, d-tile-major) and xN bf16 [128, NS*D] (DVE cast, s-tile-major).

The per-q-chunk epilogue's PSUM-freeing evictions are emitted eagerly; the
PE-side tail (1/rs row->col transposes + y matmuls) is deferred into the
next chunk's kt-loop so the PE never drains. An fp8/bf16 warmup burst at
kernel start flips the PE HAM clock-gate to 2.4 GHz while the first DMAs
are in flight.
"""

import os
from contextlib import ExitStack

import numpy as np

import concourse.bass as bass
import concourse.tile as tile
from concourse import bacc, mybir
from concourse.bass_utils import run_bass_kernel_spmd
from concourse.masks import make_identity

N_CORES = 8
B, S, D = 16, 2048, 512
BPC = B // N_CORES  # batches per core
P = 128
ND = D // P         # 4   tiles over d/e/f dims
NS = S // P         # 16  tiles over s (= q = k) dim
QC = 512            # s/q-chunk width (PSUM bank)
NQC = S // QC       # 4
TPC = QC // P       # 4   128-tiles per chunk
SCALE = float(1.0 / np.sqrt(D))
A_SC = 256.0 * SCALE   # fp8 A storage scale: Ah+Al = 256*SCALE*(Wq^T Wk)
HT_SC = 16.0           # HT fp8 tiles hold 16*(x A); exp applies 1/16
V_SC = 64.0            # v fp8 column holds 64*v; w eviction applies 1/64

F32 = mybir.dt.float32
F32R = mybir.dt.float32r
F8 = mybir.dt.float8e4
BF16 = mybir.dt.bfloat16
AFT = mybir.ActivationFunctionType
ALU = mybir.AluOpType
DR = mybir.MatmulPerfMode.DoubleRow
A_FB = False  # fp8 residual feedback on A: +~3e-3 rel err when off, -13.8us when off


def _emit(tc, x_ap, w_aps, b_aps, y_ap):
    nc = tc.nc
    ctx = ExitStack()
    with ctx:
        # ---- pools ----
        consts = ctx.enter_context(tc.tile_pool(name="consts", bufs=1))
        stage = ctx.enter_context(tc.tile_pool(name="stage", bufs=6))
        wset = ctx.enter_context(tc.tile_pool(name="wset", bufs=12))
        ab_pool = ctx.enter_context(tc.tile_pool(name="ab", bufs=1))
        xt_pool = ctx.enter_context(tc.tile_pool(name="xt", bufs=2))
        xn_pool = ctx.enter_context(tc.tile_pool(name="xn", bufs=2))
        ht_pool = ctx.enter_context(tc.tile_pool(name="ht", bufs=2))
        oc_pool = ctx.enter_context(tc.tile_pool(name="oc", bufs=3))
        at_pool = ctx.enter_context(tc.tile_pool(name="at", bufs=4))
        y_pool = ctx.enter_context(tc.tile_pool(name="y", bufs=2))
        rs_pool = ctx.enter_context(tc.tile_pool(name="rs", bufs=2))
        ppt = ctx.enter_context(tc.tile_pool(name="ppt", bufs=3, space="PSUM"))
        ppo = ctx.enter_context(tc.tile_pool(name="ppo", bufs=4, space="PSUM"))
        ppr = ctx.enter_context(tc.tile_pool(name="ppr", bufs=1, space="PSUM"))

        def pt_tile():
            return ppt.tile([P, QC], F32, tag="ppt", name="pt")

        # ---- constants ----
        ones_bf = consts.tile([P, P], BF16, tag="ones_bf")
        nc.vector.memset(ones_bf[:], 1.0)

        def filler(n=1):
            # bf16 no-op matmuls that keep the PE HAM activity window busy
            # through DMA-bound stretches so the clock gate stays at 2.4 GHz
            for _ in range(n):
                ps = pt_tile()
                nc.tensor.matmul(
                    ps[:, 0:P], ones_bf[:], ones_bf[:], start=True, stop=True
                )

        # Dense matmul burst: ~4.5us of sustained PE activity flips the PE HAM
        # clock-gate to 8/8 (2.4 GHz) while the first DMAs are in flight.
        filler(20)
        ident = consts.tile([P, P], F32, tag="ident")
        make_identity(nc, ident[:])
        ident_r = consts.tile([P, P], F32R, tag="ident_r")
        nc.vector.tensor_copy(ident_r[:], ident[:])
        ident_bf = consts.tile([P, P], BF16, tag="ident_bf")
        nc.vector.tensor_copy(ident_bf[:], ident[:])
        ones_col_bf = consts.tile([P, 1], BF16, tag="ones_col_bf")
        nc.vector.memset(ones_col_bf[:], 1.0)
        ones_row_f = consts.tile([1, P], F32, tag="ones_row_f")
        nc.vector.memset(ones_row_f[:], 1.0)
        ones_row_r = consts.tile([1, P], F32R, tag="ones_row_r")
        nc.vector.tensor_copy(ones_row_r[:], ones_row_f[:])

        def row_to_col(row_ap, dst_ap, scale=None):
            """[1, 128] bf16 SBUF row -> [128, 1] SBUF column via PE transpose.

            bf16 (single-pass weight load): a true-FP32 transpose here is a
            multi-pass FP32_HI weight load, which wedges the PE when
            interleaved with fp8 weight loads (HW hang, bisected on-device).
            """
            ps = ppt.tile([P, QC], BF16, tag="ppt", name="ptrc")
            nc.tensor.transpose(ps[:, 0:1], row_ap, ident_bf[0:1, 0:1])
            if scale is None:
                nc.vector.tensor_copy(dst_ap, ps[:, 0:1])
            else:
                nc.vector.tensor_scalar_mul(dst_ap, ps[:, 0:1], scale)

        def load_bias_row(nm):
            st = stage.tile([1, D], F32, tag="brow", name="brow")
            nc.sync.dma_start(st[:], b_aps[nm][None, :])
            return st

        def to_bf_row(row):
            st = stage.tile([1, D], BF16, tag="bfrow", name="bfrow")
            nc.vector.tensor_copy(st[:], row[0:1, :])
            return st

        def load_wnat(nm):
            """Weight, natural [row, col] layout, rounded to f32r: 4 tiles.

            One batched DMA for the whole matrix: each DMA_DIRECT2D costs
            ~600ns of issue time on the SP queue, so 4-tile transfers are
            folded into a single descriptor (row-tile-major flat dst)."""
            wst = stage.tile([P, ND * D], F32, tag="wstage", name="wst", bufs=2)
            nc.sync.dma_start(
                wst[:].rearrange("p (rt d) -> p rt d", rt=ND),
                w_aps[nm].rearrange("(rt p) d -> p rt d", p=P),
            )
            tiles = []
            for rt in range(ND):
                t = wset.tile([P, D], F32R, tag="wset", name=f"{nm}n{rt}")
                nc.vector.tensor_copy(t[:], wst[:, D * rt : D * (rt + 1)])
                tiles.append(t)
            return tiles

        # ---- one-time weight setup ----
        # Ah/Al: fp8 hi + residual of 256*SCALE*(Wq^T Wk), d-tile-major flat.
        Ah = ab_pool.tile([P, ND * D], F8, tag="Ah", name="Ah")
        Al = ab_pool.tile([P, ND * D], F8, tag="Al", name="Al")
        Bm = ab_pool.tile([P, ND * D], BF16, tag="Bm", name="Bm")
        v_col = consts.tile([P, ND], F8, tag="v_col")
        w_setup = {}

        def setup_part1(wq, wk):
            # A = Wq^T Wk ;  v = (Wk^T bq) * SCALE
            bq_row = load_bias_row("bq")
            for dt_ in range(ND):
                ps = pt_tile()
                for et in range(ND):
                    nc.tensor.matmul(
                        ps[:],
                        wq[et][:, P * dt_ : P * (dt_ + 1)],
                        wk[et][:],
                        start=(et == 0),
                        stop=(et == ND - 1),
                    )
                sl = slice(D * dt_, D * (dt_ + 1))
                nc.vector.tensor_scalar_mul(Ah[:, sl], ps[:], A_SC)
                if A_FB:
                    nc.vector.scalar_tensor_tensor(
                        Al[:, sl], ps[:], A_SC, Ah[:, sl],
                        op0=ALU.mult, op1=ALU.subtract,
                    )
            bq_col = consts.tile([P, ND], F32R, tag="bq_col")
            bq_bf = to_bf_row(bq_row)
            for t in range(ND):
                row_to_col(bq_bf[0:1, P * t : P * (t + 1)], bq_col[:, t : t + 1])
            psv = pt_tile()
            for et in range(ND):
                nc.tensor.matmul(
                    psv[0:1, :],
                    bq_col[:, et : et + 1],
                    wk[et][:],
                    start=(et == 0),
                    stop=(et == ND - 1),
                )
            v_row = stage.tile([1, D], BF16, tag="vrow", name="v_row")
            nc.vector.tensor_scalar_mul(v_row[:], psv[0:1, :], SCALE * V_SC)
            for t in range(ND):
                row_to_col(v_row[0:1, P * t : P * (t + 1)], v_col[:, t : t + 1])

        def setup_part2(wv, wo):
            # B = Wv^T Wo^T (bf16) ;  c = bv Wo^T + bo  (broadcast to 128 rows)
            woT = [
                wset.tile([P, D], F32R, tag="wset", name=f"WoT{j}")
                for j in range(ND)
            ]
            for gt in range(ND):
                for ft in range(ND):
                    ps = ppt.tile([P, QC], F32R, tag="ppt", name="ptw")
                    nc.tensor.transpose(
                        ps[:, 0:P],
                        wo[gt][:, P * ft : P * (ft + 1)],
                        ident_r[:],
                    )
                    nc.vector.tensor_copy(woT[ft][:, P * gt : P * (gt + 1)], ps[:, 0:P])
            for dt_ in range(ND):
                ps = pt_tile()
                for ft in range(ND):
                    nc.tensor.matmul(
                        ps[:],
                        wv[ft][:, P * dt_ : P * (dt_ + 1)],
                        woT[ft][:],
                        start=(ft == 0),
                        stop=(ft == ND - 1),
                    )
                nc.vector.tensor_copy(Bm[:, D * dt_ : D * (dt_ + 1)], ps[:])
            bv_row = load_bias_row("bv")
            bo_row = load_bias_row("bo")
            bv_col = stage.tile([P, ND], F32R, tag="bvcol", name="bv_col")
            bv_bf = to_bf_row(bv_row)
            for t in range(ND):
                row_to_col(bv_bf[0:1, P * t : P * (t + 1)], bv_col[:, t : t + 1])
            psc = pt_tile()
            for ft in range(ND):
                nc.tensor.matmul(
                    psc[0:1, :],
                    bv_col[:, ft : ft + 1],
                    woT[ft][:],
                    start=(ft == 0),
                    stop=(ft == ND - 1),
                )
            c_row = stage.tile([1, D], F32R, tag="crow", name="c_row")
            nc.vector.tensor_add(c_row[:], psc[0:1, :], bo_row[0:1, :])
            psb = pt_tile()
            nc.tensor.matmul(psb[:], ones_row_r[:], c_row[:], start=True, stop=True)
            c_bc = consts.tile([P, D], F32, tag="c_bc")
            nc.vector.tensor_copy(c_bc[:], psb[:])
            w_setup["c_bc"] = c_bc

        # per-q-chunk epilogue. The PSUM-freeing evictions (ZT chunk -> SBUF
        # bf16, rowsum -> SBUF) are emitted immediately at chunk end; the
        # PE-side tail (1/rs transposes + y projection) is deferred into the
        # next chunk's kt-loop so the PE never drains between chunks.
        state = {"pending": None}

        def evict_chunk(b, qc, po, pr):
            rsrow = rs_pool.tile([1, QC], BF16, tag="rs", name="rsrow")
            nc.vector.tensor_copy(rsrow[:], pr[0:1, :])
            oc = oc_pool.tile([P, ND * QC], BF16, tag="oc", name="oc")
            for dt_ in range(ND):
                sl = slice(QC * dt_, QC * (dt_ + 1))
                if dt_ == 1:
                    nc.scalar.activation(oc[:, sl], po[dt_][:], AFT.Copy)
                else:
                    nc.vector.tensor_copy(oc[:, sl], po[dt_][:])
            return (b, qc, oc, rsrow)

        def emit_epilogue(b, qc, oc, rsrow):
            rsT = rs_pool.tile([P, TPC], F32, tag="rsT", name="rsT")
            for j in range(TPC):
                row_to_col(rsrow[0:1, P * j : P * (j + 1)], rsT[:, j : j + 1])
            rsr = rs_pool.tile([P, TPC], F32, tag="rsr", name="rsr")
            nc.vector.reciprocal(rsr[:], rsT[:])
            # all 4 q-tiles accumulate into one flat SBUF tile; a single
            # batched DMA (issued from the otherwise-idle gpsimd queue, off
            # the busy SP queue) writes the whole q-chunk back.
            ysb = y_pool.tile([P, TPC * D], F32, tag="y", name="ysb")
            for j in range(TPC):
                ps = pt_tile()
                for dt_ in range(ND):
                    nc.tensor.matmul(
                        ps[:],
                        oc[:, QC * dt_ + P * j : QC * dt_ + P * (j + 1)],
                        Bm[:, D * dt_ : D * (dt_ + 1)],
                        start=(dt_ == 0),
                        stop=(dt_ == ND - 1),
                    )
                nc.vector.scalar_tensor_tensor(
                    ysb[:, D * j : D * (j + 1)],
                    ps[:],
                    rsr[:, j : j + 1],
                    w_setup["c_bc"][:],
                    op0=ALU.mult,
                    op1=ALU.add,
                )
            nc.gpsimd.dma_start(
                y_ap[b, QC * qc : QC * (qc + 1), :].rearrange(
                    "(j p) d -> p j d", p=P
                ),
                ysb[:].rearrange("p (j d) -> p j d", j=TPC),
            )

        # ---- per batch residents ----
        # xT8: one flat fp8 [128, ND*S] tile per batch, d-tile-major: column
        # block dt*S + s holds x[s, dt*128+p]. One strided DVE copy evicts a
        # whole x-tile's 4 transposed blocks at once.
        # xN: one flat bf16 [128, NS*D] tile per batch, s-tile-major: column
        # block i*D + d holds x[i*128+p, d].
        xTs = [
            xt_pool.tile([P, ND * S], F8, tag="xt", name=f"xT{b}")
            for b in range(BPC)
        ]
        xNs = [
            xn_pool.tile([P, NS * D], BF16, tag="xn", name=f"xN{b}")
            for b in range(BPC)
        ]
        chunks_done = [set() for _ in range(BPC)]

        def xt3(bb):
            return xTs[bb][:].rearrange("p (dt s) -> p dt s", dt=ND)

        def emit_x_chunk(bb, sc):
            # DMA (one batched descriptor) + bf16-cast + fp8-transpose one
            # 512-wide s-chunk of batch bb
            chunks_done[bb].add(sc)
            st = stage.tile([P, TPC * D], F32R, tag="xstage", name="xst", bufs=3)
            nc.sync.dma_start(
                st[:].rearrange("p (j d) -> p j d", j=TPC),
                x_ap[bb, QC * sc : QC * (sc + 1), :]
                .rearrange("(j p) d -> p j d", p=P)
                .bitcast(F32R),
            )
            for j in range(TPC):
                i = TPC * sc + j
                nc.vector.tensor_copy(
                    xNs[bb][:, D * i : D * (i + 1)], st[:, D * j : D * (j + 1)]
                )
                ps = ppt.tile([P, QC], BF16, tag="ppt", name="ptr")
                for dt_ in range(ND):
                    nc.tensor.transpose(
                        ps[:, P * dt_ : P * (dt_ + 1)],
                        xNs[bb][:, D * i + P * dt_ : D * i + P * (dt_ + 1)],
                        ident_bf[:],
                    )
                nc.vector.tensor_copy(
                    xt3(bb)[:, :, P * i : P * (i + 1)],
                    ps[:].rearrange("p (dt c) -> p dt c", dt=ND),
                )

        wrows = {}
        wcols = {}

        def emit_w_chunk(bb, sc):
            # w[k] = x . v for one 512-wide k-chunk (plain fp8 matmuls;
            # DoubleRow's pair-stride constraint rules out the 1-byte-stride
            # v column pairs), evicted to a bf16 row then PE-transposed into
            # the per-batch w column. Emitted as soon as chunk sc's xT lands
            # so the first exp (which needs the whole w column) never waits
            # on the last chunk's DMA.
            if bb not in wcols:
                wcols[bb] = rs_pool.tile([P, NS], F32, tag="w_col", name="w_col")
                wrows[bb] = rs_pool.tile(
                    [1, S], BF16, tag="w_row", name="w_row", bufs=2
                )
            w_row = wrows[bb]
            psw = pt_tile()
            for dt_ in range(ND):
                nc.tensor.matmul(
                    psw[0:1, :],
                    v_col[:, dt_ : dt_ + 1],
                    xTs[bb][:, S * dt_ + QC * sc : S * dt_ + QC * (sc + 1)],
                    start=(dt_ == 0),
                    stop=(dt_ == ND - 1),
                )
            nc.vector.tensor_scalar_mul(
                w_row[0:1, QC * sc : QC * (sc + 1)], psw[0:1, :], 1.0 / V_SC
            )
            for j in range(TPC):
                i = TPC * sc + j
                row_to_col(
                    w_row[0:1, P * i : P * (i + 1)], wcols[bb][:, i : i + 1]
                )

        for b in range(BPC):
            HT = [None] * NQC  # per-q-chunk flat fp8 [128, ND*QC], computed JIT
            for sc in range(NQC):
                if b == 0 and sc == 0:
                    # Wq/Wk DMAs go out first: A = Wq^T Wk heads the longest
                    # dependency chain (A -> HT(0) -> attention)
                    wsetup = getattr(_emit, "_ws", {})
                    _emit._ws = wsetup
                    wsetup["wq"] = load_wnat("Wq")
                    wsetup["wk"] = load_wnat("Wk")
                if b == 0 and sc == 1:
                    # A/v setup runs BEFORE chunk1's transposes hit the PE
                    # queue: A = Wq^T Wk heads the critical chain to the
                    # first scores block, and only needs the Wq/Wk casts.
                    wsetup = _emit._ws
                    setup_part1(wsetup.pop("wq"), wsetup.pop("wk"))
                    wsetup["wv"] = load_wnat("Wv")
                    wsetup["wo"] = load_wnat("Wo")
                if sc not in chunks_done[b]:
                    emit_x_chunk(b, sc)
                if b == 0:
                    if sc == 1:
                        emit_w_chunk(0, 0)
                        emit_w_chunk(0, 1)
                    elif sc > 1:
                        emit_w_chunk(0, sc)

            def emit_ht(hsc):
                # HT chunk hsc: fp8 flat [128, ND*QC] holding 16*(x A), from
                # fp8 DoubleRow matmuls over (Ah + Al residual) pairs. JIT,
                # from inside the previous chunk's kt-loop so the PE stream
                # stays dense.
                HT[hsc] = ht_pool.tile([P, ND * QC], F8, tag="ht", name="HT")
                xts = xt3(b)
                ah3 = Ah[:].rearrange("p (dt e) -> p dt e", dt=ND)
                al3 = Al[:].rearrange("p (dt e) -> p dt e", dt=ND)
                a3s = (ah3, al3) if A_FB else (ah3,)
                nmm = 2 * len(a3s)
                for dpt in range(ND):
                    ps = pt_tile()
                    k = 0
                    for a3 in a3s:
                        for jp in range(2):
                            nc.tensor.matmul(
                                ps[:],
                                a3[:, 2 * jp : 2 * jp + 2, P * dpt : P * (dpt + 1)],
                                xts[:, 2 * jp : 2 * jp + 2, QC * hsc : QC * (hsc + 1)],
                                start=(k == 0),
                                stop=(k == nmm - 1),
                                perf_mode=DR,
                            )
                            k += 1
                    nc.scalar.activation(
                        HT[hsc][:, QC * dpt : QC * (dpt + 1)], ps[:],
                        AFT.Identity, scale=1.0 / HT_SC,
                    )

            emit_ht(0)
            # kt-PAIR loop: the PE stream alternates between one contiguous
            # fp8-DR block (scores for the NEXT pair, + JIT HT) and one
            # contiguous bf16 block (po/pr for the current pair, + deferred
            # epilogue). Mode switches (DR<->bf16 weight-load reconfig) cost
            # ~100ns each on HW; batching halves them vs per-kt alternation.
            # The rowsum matmul uses full 128-col ones weights into a full
            # [128, QC] PSUM bank (all rows identical): 1-col weight loads
            # stall the PE weight-load pipeline ~110ns every time.
            for qc in range(NQC):
                po = [
                    ppo.tile([P, QC], F32, tag="ppo", name="po") for _ in range(ND)
                ]
                pr = ppr.tile([P, QC], F32, tag="ppr", name="pr")
                pss = [None] * NS
                at = [None] * NS

                def scores(kt):
                    ps = pt_tile()
                    ht3 = HT[qc][:].rearrange("p (dpt q) -> p dpt q", dpt=ND)
                    xts = xt3(b)
                    for jp in range(2):
                        nc.tensor.matmul(
                            ps[:],
                            xts[:, 2 * jp : 2 * jp + 2, P * kt : P * (kt + 1)],
                            ht3[:, 2 * jp : 2 * jp + 2, :],
                            start=(jp == 0),
                            stop=(jp == 1),
                            perf_mode=DR,
                        )
                    pss[kt] = ps

                def expk(kt):
                    a = at_pool.tile([P, QC], BF16, tag="at", name="at")
                    nc.scalar.activation(
                        a[:], pss[kt][:], AFT.Exp,
                        bias=wcols[b][:, kt : kt + 1], scale=1.0 / HT_SC,
                    )
                    at[kt] = a

                def po_pr(kt):
                    for dt_ in range(ND):
                        nc.tensor.matmul(
                            po[dt_][:],
                            xNs[b][:, D * kt + P * dt_ : D * kt + P * (dt_ + 1)],
                            at[kt][:],
                            start=(kt == 0),
                            stop=(kt == NS - 1),
                        )
                    nc.tensor.matmul(
                        pr[:],
                        ones_bf[:],
                        at[kt][:],
                        start=(kt == 0),
                        stop=(kt == NS - 1),
                    )

                scores(0)
                scores(1)
                for p in range(NS // 2):
                    k0 = 2 * p
                    # ACT: exp of the current pair (overlaps the PE blocks)
                    expk(k0)
                    expk(k0 + 1)
                    # fp8-DR block: next pair's scores (+ JIT HT at p==3)
                    if k0 + 2 < NS:
                        scores(k0 + 2)
                        scores(k0 + 3)
                    if p == 3 and qc + 1 < NQC:
                        emit_ht(qc + 1)
                    # bf16 block: current pair's ZT/rowsum accumulation
                    po_pr(k0)
                    po_pr(k0 + 1)
                    # deferred epilogue (bf16 y matmuls: stays in-mode)
                    if p == 1 and state["pending"] is not None:
                        emit_epilogue(*state["pending"])
                        state["pending"] = None
                    # B / c are first needed by qc0's epilogue (flushed at
                    # qc1 p==1): compute them inside qc0's dense kt-loop
                    if b == 0 and qc == 0 and p == 4:
                        wsetup = _emit._ws
                        setup_part2(wsetup.pop("wv"), wsetup.pop("wo"))
                    # prefetch ALL of the next batch's x chunks into the tail
                    # of this batch's last attention chunk (2 chunks left
                    # unprefetched previously cost a ~6.5us DMA-wait stall at
                    # every batch boundary)
                    if qc == NQC - 1 and b + 1 < BPC and p in (2, 3, 5, 6):
                        c = {2: 0, 3: 1, 5: 2, 6: 3}[p]
                        emit_x_chunk(b + 1, c)
                        emit_w_chunk(b + 1, c)
                state["pending"] = evict_chunk(b, qc, po, pr)

        if state["pending"] is not None:
            emit_epilogue(*state["pending"])
            state["pending"] = None


def build_program():
    nc = bacc.Bacc("TRN2", target_bir_lowering=False, debug=False)
    x_ap = nc.dram_tensor("x", [BPC, S, D], F32, kind="ExternalInput").ap()
    w_aps = {
        nm: nc.dram_tensor(nm, [D, D], F32, kind="ExternalInput").ap()
        for nm in ("Wq", "Wk", "Wv", "Wo")
    }
    b_aps = {
        nm: nc.dram_tensor(nm, [D], F32, kind="ExternalInput").ap()
        for nm in ("bq", "bk", "bv", "bo")
    }
    y_ap = nc.dram_tensor("y", [BPC, S, D], F32, kind="ExternalOutput").ap()
    with tile.TileContext(nc) as tc:
        _emit(tc, x_ap, w_aps, b_aps, y_ap)
    nc.compile()
    return nc


_program_cache = {}


def _get_program(fast_mm=True):
    # fast_mm retained for test.py compatibility; single fp8/bf16 program.
    if "p" not in _program_cache:
        _program_cache["p"] = build_program()
    return _program_cache["p"]


def _make_in_maps(inputs):
    arrs = {
        k: np.ascontiguousarray(np.asarray(v, dtype=np.float32))
        for k, v in inputs.items()
    }
    in_maps = []
    for core in range(N_CORES):
        m = {"x": arrs["x"][BPC * core : BPC * (core + 1)]}
        for nm in ("Wq", "Wk", "Wv", "Wo", "bq", "bk", "bv", "bo"):
            m[nm] = arrs[nm]
        in_maps.append(m)
    return in_maps


def run(inputs, fast_mm=True, trace=False):
    """Returns (y_full, BassKernelResults)."""
    nc = _get_program(fast_mm)
    in_maps = _make_in_maps(inputs)
    last_err = None
    for attempt in range(3):
        try:
            res = run_bass_kernel_spmd(nc, in_maps, list(range(N_CORES)), trace=trace)
            break
        except Exception as e:  # transient NRT device errors: retry
            last_err = e
            import time

            time.sleep(2.0 * (attempt + 1))
    else:
        raise last_err
    y = np.concatenate([r["y"] for r in res.results], axis=0)
    return np.ascontiguousarray(y.astype(np.float32)), res


def kernel(**inputs):
    y, _ = run(inputs, trace=False)
    return y
